# revision 12
# baseline (speedup 1.0000x reference)
"""nn_CPN_67740224192953 kernel: conv backbone + 7x7 heads fully on 8 trn2 cores.

Per core (2 per image, half-image each):
  - im2col built ON DEVICE: 9 strided DMAs per slab from a compact
    [3, 264, 520] zero-padded x canvas (1.65MB/core upload vs 20MB im2col)
  - backbone 3x3 conv (K=27 matmul) + relu
  - head convs for [d=s1-s0, ref_x, ref_y] via taps-as-M matmul
    (M=147 rows ordered (dy,dx,c), K=64) -> P[147, NF]
  - 7x7 shift-sum ON DEVICE: 50 shifted-slice SBUF DMAs (one per tap)
    + 2 accumulating selection matmuls -> maps [3, 16x518] per slab
  - tanh(ref + b_ref) on ACT engine, downcast to fp16
  - downloads: d_map fp32 [256,512] + ref fp16 [2,256,512] per core
    (~1.6MB/core vs ~80MB of raw tap partials)
Host: top-k ordering on d_map (sigmoid is monotone -> argsort d directly),
  loc/fourier head recomputed at the 512 detections from x patches (BLAS),
  fourier contour synthesis, 4 refinement gather iterations.
"""

import numpy as np

LAST_EXEC_NS = None
LAST_DEVICE_S = None
TIMINGS = {}

B, C_IN, H, W = 4, 3, 512, 512
C = 64
ORDER = 5
SAMPLES = 32
N_DET = 512
ITERS = 4
MARGIN = 3.0
K7 = 7
WP = W + 6            # padded f-col width 518
HALF = H // 2         # 256 rows per core
SLAB = 16             # output rows per slab
NSLAB = HALF // SLAB  # 16 slabs
FROWS = SLAB + 6      # f rows per slab (halo 3 top+bottom)
NF = FROWS * WP       # 11396 f positions per slab
NFP = NF + 6          # P width (+6 so the max tap shift stays in-bounds)
ND = SLAB * WP        # 8288 out positions per slab
CH = HALF + 8         # canvas rows 264
CW = W + 8            # canvas cols 520
NCHUNK = (NF + 511) // 512   # 23
NOCHUNK = (ND + 511) // 512  # 17


def _build_device_program():
    import concourse.bacc as bacc
    import concourse.mybir as mybir
    from concourse.tile import TileContext

    nc = bacc.Bacc("TRN2", target_bir_lowering=False, num_devices=8)
    f32 = mybir.dt.float32
    f16 = mybir.dt.float16
    xc_d = nc.dram_tensor("xc", [3, CH, CW], f32, kind="ExternalInput")
    w27_d = nc.dram_tensor("w27", [27, C], f32, kind="ExternalInput")
    wa_d = nc.dram_tensor("wa", [C, 128], f32, kind="ExternalInput")
    wb_d = nc.dram_tensor("wb", [C, 19], f32, kind="ExternalInput")
    sl_d = nc.dram_tensor("sl", [128, 3], f32, kind="ExternalInput")
    sh_d = nc.dram_tensor("sh", [19, 3], f32, kind="ExternalInput")
    br_d = nc.dram_tensor("br", [2, 1], f32, kind="ExternalInput")
    # per-core 0/1 scalars: zero f halo rows that fall outside the full image
    # (head conv zero-pads f; only the first slab of h=0 / last slab of h=1)
    mzf_d = nc.dram_tensor("mzf", [C, 1], f32, kind="ExternalInput")
    mzl_d = nc.dram_tensor("mzl", [C, 1], f32, kind="ExternalInput")
    dmap_d = nc.dram_tensor("dmap", [HALF, W], f32, kind="ExternalOutput")
    ref_d = nc.dram_tensor("ref", [2, HALF, W], f16, kind="ExternalOutput")

    with (
        TileContext(nc) as tc,
        tc.tile_pool(name="wpool", bufs=1) as wpool,
        tc.tile_pool(name="big", bufs=1) as big,
        tc.tile_pool(name="psb", bufs=2, space="PSUM") as psb,
        tc.tile_pool(name="psa", bufs=2, space="PSUM") as psa,
        tc.tile_pool(name="psh", bufs=2, space="PSUM") as psh,
        tc.tile_pool(name="pso", bufs=2, space="PSUM") as pso,
    ):
        # weights: DMA in, then re-copy on DVE so every matmul's weight dep
        # is a DVE semaphore (keeps per-matmul sync-wait count at the limit)
        w27_r = wpool.tile([27, C], f32, tag="w27r")
        wa_r = wpool.tile([C, 128], f32, tag="war")
        wb_r = wpool.tile([C, 19], f32, tag="wbr")
        sl_r = wpool.tile([128, 3], f32, tag="slr")
        sh_r = wpool.tile([19, 3], f32, tag="shr")
        br_r = wpool.tile([2, 1], f32, tag="brr")
        nc.sync.dma_start(out=w27_r[:], in_=w27_d[:, :])
        nc.sync.dma_start(out=wa_r[:], in_=wa_d[:, :])
        nc.sync.dma_start(out=wb_r[:], in_=wb_d[:, :])
        nc.sync.dma_start(out=sl_r[:], in_=sl_d[:, :])
        nc.sync.dma_start(out=sh_r[:], in_=sh_d[:, :])
        nc.sync.dma_start(out=br_r[:], in_=br_d[:, :])
        mzf_t = wpool.tile([C, 1], f32, tag="mzf")
        mzl_t = wpool.tile([C, 1], f32, tag="mzl")
        nc.sync.dma_start(out=mzf_t[:], in_=mzf_d[:, :])
        nc.sync.dma_start(out=mzl_t[:], in_=mzl_d[:, :])
        w27_t = wpool.tile([27, C], f32, tag="w27")
        wa_t = wpool.tile([C, 128], f32, tag="wa")
        wb_t = wpool.tile([C, 19], f32, tag="wb")
        sl_t = wpool.tile([128, 3], f32, tag="sl")
        sh_t = wpool.tile([19, 3], f32, tag="sh")
        br_t = wpool.tile([2, 1], f32, tag="br")
        nc.vector.tensor_copy(w27_t[:], w27_r[:])
        nc.vector.tensor_copy(wa_t[:], wa_r[:])
        nc.vector.tensor_copy(wb_t[:], wb_r[:])
        nc.vector.tensor_copy(sl_t[:], sl_r[:])
        nc.vector.tensor_copy(sh_t[:], sh_r[:])
        nc.vector.tensor_copy(br_t[:], br_r[:])

        for s in range(NSLAB):
            r0 = s * SLAB
            # --- im2col straight from the DRAM canvas (row order ky,kx,cin) ---
            imc = big.tile([27, FROWS, WP], f32, tag="imc")
            for ky in range(3):
                for kx in range(3):
                    g = ky * 3 + kx
                    nc.sync.dma_start(
                        out=imc[3 * g:3 * g + 3, :, :],
                        in_=xc_d[:, r0 + ky:r0 + ky + FROWS, kx:kx + WP])
            imcF = imc.rearrange("p a b -> p (a b)")
            # --- backbone: f = relu(w27.T @ imc) ---
            f_t = big.tile([64, NF], f32, tag="f")
            for k in range(NCHUNK):
                a, b = k * 512, min((k + 1) * 512, NF)
                pbb = psb.tile([64, 512], f32, tag="pbb")
                nc.tensor.matmul(out=pbb[:, :b - a], lhsT=w27_t[:],
                                 rhs=imcF[:, a:b], start=True, stop=True)
                nc.scalar.activation(f_t[:, a:b], pbb[:, :b - a],
                                     mybir.ActivationFunctionType.Relu)
            # head conv pads f with zeros beyond the image's column range
            f3 = f_t.rearrange("p (a b) -> p a b", a=FROWS)
            nc.vector.memset(f3[:, :, 0:3], 0.0)
            nc.vector.memset(f3[:, :, WP - 3:WP], 0.0)
            # ... and beyond the image's row range (per-core 0/1 scalar)
            if s == 0:
                nc.vector.tensor_scalar_mul(
                    f_t[:, 0:3 * WP], f_t[:, 0:3 * WP], mzf_t[:])
            if s == NSLAB - 1:
                nc.vector.tensor_scalar_mul(
                    f_t[:, NF - 3 * WP:NF], f_t[:, NF - 3 * WP:NF], mzl_t[:])
            # --- heads: P[(dy,dx,c), pos] = w147.T @ f ---
            plo = big.tile([128, NFP], f32, tag="plo")
            phi = big.tile([19, NFP], f32, tag="phi")
            nc.vector.memset(plo[:, NF:NFP], 0.0)
            nc.vector.memset(phi[:, NF:NFP], 0.0)
            for k in range(NCHUNK):
                a, b = k * 512, min((k + 1) * 512, NF)
                pa = psa.tile([128, 512], f32, tag="pa")
                pb = psh.tile([19, 512], f32, tag="pb")
                nc.tensor.matmul(out=pa[:, :b - a], lhsT=wa_t[:],
                                 rhs=f_t[:, a:b], start=True, stop=True)
                nc.tensor.matmul(out=pb[:, :b - a], lhsT=wb_t[:],
                                 rhs=f_t[:, a:b], start=True, stop=True)
                nc.vector.tensor_copy(plo[:, a:b], pa[:, :b - a])
                nc.scalar.copy(phi[:, a:b], pb[:, :b - a])
            # --- tap shift-copies: Psh[r, o] = P[r, o + dy*WP + dx] ---
            pshlo = big.tile([128, ND], f32, tag="imc")
            pshhi = big.tile([19, ND], f32, tag="f")
            for t in range(49):
                sft = (t // 7) * WP + (t % 7)
                lo, hi = 3 * t, 3 * t + 3
                if hi <= 128:
                    nc.sync.dma_start(out=pshlo[lo:hi, :],
                                      in_=plo[lo:hi, sft:sft + ND])
                elif lo >= 128:
                    nc.sync.dma_start(out=pshhi[lo - 128:hi - 128, :],
                                      in_=phi[lo - 128:hi - 128, sft:sft + ND])
                else:
                    nc.sync.dma_start(out=pshlo[lo:128, :],
                                      in_=plo[lo:128, sft:sft + ND])
                    nc.sync.dma_start(out=pshhi[0:hi - 128, :],
                                      in_=phi[0:hi - 128, sft:sft + ND])
            # --- selection matmuls: maps[3, o] = sum_r Sel[r,c] Psh[r, o] ---
            out_t = big.tile([3, ND], f32, tag="phi")
            for k in range(NOCHUNK):
                a, b = k * 512, min((k + 1) * 512, ND)
                po = pso.tile([3, 512], f32, tag="po")
                nc.tensor.matmul(out=po[:, :b - a], lhsT=sl_t[:],
                                 rhs=pshlo[:, a:b], start=True, stop=False)
                nc.tensor.matmul(out=po[:, :b - a], lhsT=sh_t[:],
                                 rhs=pshhi[:, a:b], start=False, stop=True)
                nc.vector.tensor_copy(out_t[:, a:b], po[:, :b - a])
            # --- tanh(ref + b_ref) -> fp16, dump maps ---
            # (channel order is ref_x, ref_y, d: ACT input APs must start
            # at partition 0, so the tanh rows lead)
            tanh_t = big.tile([2, ND], f16, tag="imc")
            nc.scalar.activation(tanh_t[:, :], out_t[0:2, :],
                                 mybir.ActivationFunctionType.Tanh,
                                 bias=br_t[:])
            out3 = out_t.rearrange("p (a b) -> p a b", a=SLAB)
            tanh3 = tanh_t.rearrange("p (a b) -> p a b", a=SLAB)
            nc.sync.dma_start(out=dmap_d[r0:r0 + SLAB, :],
                              in_=out3[2:3, :, 0:W])
            nc.sync.dma_start(out=ref_d[:, r0:r0 + SLAB, :],
                              in_=tanh3[:, :, 0:W])
    nc.finalize()
    return nc


def _host_canvases(x):
    """Per (image, half): zero-padded canvas [3, CH, CW]; canvas[c, u, v]
    = x[c, h*HALF + u - 4, v - 4] where in-bounds, else 0."""
    xg = np.zeros((B, C_IN, H + 8, W + 8), np.float32)
    xg[:, :, 4:4 + H, 4:4 + W] = x
    out = {}
    for b in range(B):
        for h in range(2):
            out[(b, h)] = np.ascontiguousarray(
                xg[b, :, h * HALF:h * HALF + CH, :])
    return out


_NC_CACHE = None


def kernel(x, w_bb, b_bb, w_score, b_score, w_loc, b_loc,
           w_fourier, b_fourier, w_ref, b_ref):
    import time as _time
    global LAST_EXEC_NS, LAST_DEVICE_S, _NC_CACHE
    t_start = _time.time()
    x = np.asarray(x, np.float32)
    w_bb = np.asarray(w_bb, np.float32)
    w_score = np.asarray(w_score, np.float32)
    w_loc = np.asarray(w_loc, np.float32)
    w_fourier = np.asarray(w_fourier, np.float32)
    w_ref = np.asarray(w_ref, np.float32)
    b_bb = np.asarray(b_bb, np.float32)
    b_ref_np = np.asarray(b_ref, np.float32)

    # ---- weights prep ----
    # backbone lhsT rows ordered (ky, kx, cin) to match the im2col DMAs
    w27 = np.ascontiguousarray(w_bb.transpose(2, 3, 1, 0).reshape(27, C))
    w_d = (w_score[1] - w_score[0]).astype(np.float32)          # [C,7,7]
    whead = np.stack([w_ref[0], w_ref[1], w_d], 0)              # [3,C,7,7]
    w147T = np.ascontiguousarray(
        whead.transpose(2, 3, 0, 1).reshape(147, C))            # rows (dy,dx,c)
    wa = np.ascontiguousarray(w147T[:128].T)                    # [C,128]
    wb = np.ascontiguousarray(w147T[128:].T)                    # [C,19]
    rr = np.arange(147)
    sel = (rr[:, None] % 3 == np.arange(3)[None, :]).astype(np.float32)
    sl = np.ascontiguousarray(sel[:128])
    sh = np.ascontiguousarray(sel[128:])
    br = np.ascontiguousarray(b_ref_np.reshape(2, 1))

    canv = _host_canvases(x)
    TIMINGS['prep'] = _time.time() - t_start

    # ---- device run ----
    from concourse.bass_utils import run_bass_kernel_spmd
    t0 = _time.time()
    if _NC_CACHE is None:
        _NC_CACHE = _build_device_program()
    nc = _NC_CACHE
    TIMINGS['build'] = _time.time() - t0
    mz0 = np.zeros((C, 1), np.float32)
    mz1 = np.ones((C, 1), np.float32)
    in_maps = []
    for core in range(8):
        b, h = core // 2, core % 2
        in_maps.append({"xc": canv[(b, h)], "w27": w27, "wa": wa, "wb": wb,
                        "sl": sl, "sh": sh, "br": br,
                        "mzf": mz0 if h == 0 else mz1,
                        "mzl": mz1 if h == 0 else mz0})
    t0 = _time.time()
    res = run_bass_kernel_spmd(nc, in_maps, core_ids=list(range(8)))
    LAST_DEVICE_S = _time.time() - t0
    TIMINGS['device'] = LAST_DEVICE_S
    LAST_EXEC_NS = res.exec_time_ns

    # ---- host: assemble maps ----
    t0 = _time.time()
    d_map = np.empty((B, H, W), np.float32)
    ref_map = np.empty((B, 2, H, W), np.float32)
    for core in range(8):
        b, h = core // 2, core % 2
        sl_ = slice(h * HALF, (h + 1) * HALF)
        d_map[b, sl_] = res.results[core]["dmap"]
        ref_map[b, :, sl_] = np.float32(MARGIN) * np.asarray(
            res.results[core]["ref"], np.float32)

    # ---- top-k ordering: sigmoid is monotone in d, so order by d directly
    # (value desc, index asc on ties — matches jax.lax.top_k) ----
    dd = d_map.reshape(B, H * W)
    NC_ = 2048
    cand = np.argpartition(-dd, NC_ - 1, axis=1)[:, :NC_]
    cval = np.take_along_axis(dd, cand, 1)
    top_idx = np.empty((B, N_DET), np.int32)
    for b in range(B):
        order = np.lexsort((cand[b], -cval[b]))[:N_DET]
        top_idx[b] = cand[b][order]
    px = (top_idx % W).astype(np.float32)
    py = (top_idx // W).astype(np.float32)

    # ---- loc/fourier head at detections: recompute f patches from x ----
    xp = np.zeros((B, C_IN, H + 8, W + 8), np.float32)
    xp[:, :, 4:4 + H, 4:4 + W] = x
    swx = np.lib.stride_tricks.sliding_window_view(xp, (9, 9), axis=(2, 3))
    # swx[b, c, iy, ix, :, :] = xp[b, c, iy:iy+9, ix:ix+9]
    w27h = w_bb.reshape(C, 27).T                                # rows (cin,ky,kx)
    w22 = np.concatenate([w_loc, w_fourier], 0)                 # [22,C,7,7]
    w22f = w22.reshape(22, C * 49)
    b22 = np.concatenate([np.asarray(b_loc, np.float32),
                          np.asarray(b_fourier, np.float32)], 0)
    head22 = np.empty((B, N_DET, 22), np.float32)
    for b in range(B):
        iy, ix = top_idx[b] // W, top_idx[b] % W
        pat = swx[b][:, iy, ix]                                 # [3,N,9,9]
        pw = np.lib.stride_tricks.sliding_window_view(
            pat, (3, 3), axis=(2, 3))                           # [3,N,7,7,3,3]
        pwr = np.ascontiguousarray(
            pw.transpose(1, 2, 3, 0, 4, 5)).reshape(N_DET * 49, 27)
        fw = np.maximum(pwr @ w27h + b_bb[None, :], 0.0)        # [N*49, C]
        # head conv zero-pads f beyond the image: mask out-of-bounds positions
        off = np.arange(7) - 3
        fy = iy[:, None, None] + off[:, None]                   # [N,7,1]
        fx = ix[:, None, None] + off[None, :]                   # [N,1,7]
        valid = ((fy >= 0) & (fy < H) & (fx >= 0) & (fx < W))   # [N,7,7]
        fw = fw.reshape(N_DET, 49, C) * valid.reshape(N_DET, 49, 1)
        fwr = fw.transpose(0, 2, 1).reshape(N_DET, C * 49)
        head22[b] = fwr @ w22f.T + b22[None, :]

    loc = head22[..., 0:2]
    coef = head22[..., 2:22].reshape(B, N_DET, ORDER, 4)
    cx = (px + loc[..., 0]).astype(np.float32)
    cy = (py + loc[..., 1]).astype(np.float32)

    # ---- fourier contour synthesis ----
    t = np.arange(SAMPLES, dtype=np.float32) / np.float32(SAMPLES)
    kk = np.arange(1, ORDER + 1, dtype=np.float32)
    ang = (np.float32(2.0 * np.pi) * kk[:, None] * t[None, :]).astype(np.float32)
    cos_a = np.cos(ang).astype(np.float32)
    sin_a = np.sin(ang).astype(np.float32)
    xs = (np.einsum("bno,os->bns", coef[..., 0], cos_a, dtype=np.float32)
          + np.einsum("bno,os->bns", coef[..., 1], sin_a, dtype=np.float32)
          + cx[..., None]).astype(np.float32)
    ys = (np.einsum("bno,os->bns", coef[..., 2], cos_a, dtype=np.float32)
          + np.einsum("bno,os->bns", coef[..., 3], sin_a, dtype=np.float32)
          + cy[..., None]).astype(np.float32)
    det = np.stack([xs, ys], -1)

    # ---- refinement iterations ----
    ref_flat = ref_map.reshape(B, 2, H * W)
    for _ in range(ITERS):
        deti = np.round(det)
        xc_ = np.clip(deti[..., 0], 0, W - 1)
        yc_ = np.clip(deti[..., 1], 0, H - 1)
        lin = (yc_.astype(np.int32) * W + xc_.astype(np.int32)).reshape(B, N_DET * SAMPLES)
        rx = np.take_along_axis(ref_flat[:, 0], lin, 1).reshape(B, N_DET, SAMPLES)
        ry = np.take_along_axis(ref_flat[:, 1], lin, 1).reshape(B, N_DET, SAMPLES)
        det = np.stack([(xc_ + rx).astype(np.float32),
                        (yc_ + ry).astype(np.float32)], -1)
    TIMINGS['post'] = _time.time() - t0
    TIMINGS['total'] = _time.time() - t_start
    return det.astype(np.float32)


# revision 14
# speedup vs baseline: 1.2888x; 1.2888x over previous
"""nn_CPN_67740224192953 kernel: conv backbone + 7x7 heads fully on 8 trn2 cores.

Per core (2 per image, half-image each):
  - im2col built ON DEVICE: 9 strided DMAs per slab from a compact
    [3, 264, 520] zero-padded x canvas (1.65MB/core upload vs 20MB im2col)
  - backbone 3x3 conv (K=27 matmul) + relu
  - head convs for [d=s1-s0, ref_x, ref_y] via taps-as-M matmul
    (M=147 rows ordered (dy,dx,c), K=64) -> P[147, NF]
  - 7x7 shift-sum ON DEVICE: 50 shifted-slice SBUF DMAs (one per tap)
    + 2 accumulating selection matmuls -> maps [3, 16x518] per slab
  - tanh(ref + b_ref) on ACT engine, downcast to fp16
  - downloads: d_map fp32 [256,512] + ref fp16 [2,256,512] per core
    (~1.6MB/core vs ~80MB of raw tap partials)
Host: top-k ordering on d_map (sigmoid is monotone -> argsort d directly),
  loc/fourier head recomputed at the 512 detections from x patches (BLAS),
  fourier contour synthesis, 4 refinement gather iterations.
"""

import numpy as np

LAST_EXEC_NS = None
LAST_DEVICE_S = None
TIMINGS = {}

B, C_IN, H, W = 4, 3, 512, 512
C = 64
ORDER = 5
SAMPLES = 32
N_DET = 512
ITERS = 4
MARGIN = 3.0
K7 = 7
WP = W + 6            # padded f-col width 518
HALF = H // 2         # 256 rows per core
SLAB = 16             # output rows per slab
NSLAB = HALF // SLAB  # 16 slabs
FROWS = SLAB + 6      # f rows per slab (halo 3 top+bottom)
NF = FROWS * WP       # 11396 f positions per slab
NFP = NF + 6          # P width (+6 so the max tap shift stays in-bounds)
ND = SLAB * WP        # 8288 out positions per slab
CH = HALF + 8         # canvas rows 264
CW = W + 8            # canvas cols 520
NCHUNK = (NF + 511) // 512   # 23
NOCHUNK = (ND + 511) // 512  # 17


def _build_device_program():
    import concourse.bacc as bacc
    import concourse.bass as bass
    import concourse.mybir as mybir
    from concourse.tile import TileContext

    nc = bacc.Bacc("TRN2", target_bir_lowering=False, num_devices=8)
    f32 = mybir.dt.float32
    f16 = mybir.dt.float16
    xc_d = nc.dram_tensor("xc", [3, CH, CW], f32, kind="ExternalInput")
    w27_d = nc.dram_tensor("w27", [27, C], f32, kind="ExternalInput")
    wa_d = nc.dram_tensor("wa", [C, 128], f32, kind="ExternalInput")
    wb_d = nc.dram_tensor("wb", [C, 19], f32, kind="ExternalInput")
    sl_d = nc.dram_tensor("sl", [128, 3], f32, kind="ExternalInput")
    sh_d = nc.dram_tensor("sh", [19, 3], f32, kind="ExternalInput")
    br_d = nc.dram_tensor("br", [2, 1], f32, kind="ExternalInput")
    # per-core 0/1 scalars: zero f halo rows that fall outside the full image
    # (head conv zero-pads f; only the first slab of h=0 / last slab of h=1)
    mzf_d = nc.dram_tensor("mzf", [C, 1], f32, kind="ExternalInput")
    mzl_d = nc.dram_tensor("mzl", [C, 1], f32, kind="ExternalInput")
    dmap_d = nc.dram_tensor("dmap", [HALF, W], f32, kind="ExternalOutput")
    ref_d = nc.dram_tensor("ref", [2, HALF, W], f16, kind="ExternalOutput")

    with (
        TileContext(nc) as tc,
        tc.tile_pool(name="wpool", bufs=1) as wpool,
        tc.tile_pool(name="big", bufs=1) as big,
        tc.tile_pool(name="psb", bufs=2, space="PSUM") as psb,
        tc.tile_pool(name="psa", bufs=2, space="PSUM") as psa,
        tc.tile_pool(name="psh", bufs=2, space="PSUM") as psh,
        tc.tile_pool(name="pso", bufs=2, space="PSUM") as pso,
    ):
        # weights: DMA in, then re-copy on DVE so every matmul's weight dep
        # is a DVE semaphore (keeps per-matmul sync-wait count at the limit)
        w27_r = wpool.tile([27, C], f32, tag="w27r")
        wa_r = wpool.tile([C, 128], f32, tag="war")
        wb_r = wpool.tile([C, 19], f32, tag="wbr")
        sl_r = wpool.tile([128, 3], f32, tag="slr")
        sh_r = wpool.tile([19, 3], f32, tag="shr")
        br_r = wpool.tile([2, 1], f32, tag="brr")
        nc.sync.dma_start(out=w27_r[:], in_=w27_d[:, :])
        nc.sync.dma_start(out=wa_r[:], in_=wa_d[:, :])
        nc.sync.dma_start(out=wb_r[:], in_=wb_d[:, :])
        nc.sync.dma_start(out=sl_r[:], in_=sl_d[:, :])
        nc.sync.dma_start(out=sh_r[:], in_=sh_d[:, :])
        nc.sync.dma_start(out=br_r[:], in_=br_d[:, :])
        mzf_t = wpool.tile([C, 1], f32, tag="mzf")
        mzl_t = wpool.tile([C, 1], f32, tag="mzl")
        nc.sync.dma_start(out=mzf_t[:], in_=mzf_d[:, :])
        nc.sync.dma_start(out=mzl_t[:], in_=mzl_d[:, :])
        w27_t = wpool.tile([27, C], f32, tag="w27")
        wa_t = wpool.tile([C, 128], f32, tag="wa")
        wb_t = wpool.tile([C, 19], f32, tag="wb")
        sl_t = wpool.tile([128, 3], f32, tag="sl")
        sh_t = wpool.tile([19, 3], f32, tag="sh")
        br_t = wpool.tile([2, 1], f32, tag="br")
        nc.vector.tensor_copy(w27_t[:], w27_r[:])
        nc.vector.tensor_copy(wa_t[:], wa_r[:])
        nc.vector.tensor_copy(wb_t[:], wb_r[:])
        nc.vector.tensor_copy(sl_t[:], sl_r[:])
        nc.vector.tensor_copy(sh_t[:], sh_r[:])
        nc.vector.tensor_copy(br_t[:], br_r[:])

        def emit_slab(r0, first, last):
            """One 16-row slab. r0 is a python int (peeled edge slabs) or a
            For_i loop value (middle slabs)."""
            dyn = not isinstance(r0, int)
            # --- im2col straight from the DRAM canvas (row order ky,kx,cin) ---
            imc = big.tile([27, FROWS, WP], f32, tag="imc", name="imc")
            for ky in range(3):
                for kx in range(3):
                    g = ky * 3 + kx
                    rsl = (bass.ds(r0 + ky, FROWS) if dyn
                           else slice(r0 + ky, r0 + ky + FROWS))
                    nc.sync.dma_start(
                        out=imc[3 * g:3 * g + 3, :, :],
                        in_=xc_d[:, rsl, kx:kx + WP])
            imcF = imc.rearrange("p a b -> p (a b)")
            # --- backbone: f = relu(w27.T @ imc) ---
            f_t = big.tile([64, NF], f32, tag="f", name="f_t")
            for k in range(NCHUNK):
                a, b = k * 512, min((k + 1) * 512, NF)
                pbb = psb.tile([64, 512], f32, tag="pbb", name="pbb")
                nc.tensor.matmul(out=pbb[:, :b - a], lhsT=w27_t[:],
                                 rhs=imcF[:, a:b], start=True, stop=True)
                nc.scalar.activation(f_t[:, a:b], pbb[:, :b - a],
                                     mybir.ActivationFunctionType.Relu)
            # head conv pads f with zeros beyond the image's column range
            f3 = f_t.rearrange("p (a b) -> p a b", a=FROWS)
            nc.vector.memset(f3[:, :, 0:3], 0.0)
            nc.vector.memset(f3[:, :, WP - 3:WP], 0.0)
            # ... and beyond the image's row range (per-core 0/1 scalar)
            if first:
                nc.vector.tensor_scalar_mul(
                    f_t[:, 0:3 * WP], f_t[:, 0:3 * WP], mzf_t[:])
            if last:
                nc.vector.tensor_scalar_mul(
                    f_t[:, NF - 3 * WP:NF], f_t[:, NF - 3 * WP:NF], mzl_t[:])
            # --- heads: P[(dy,dx,c), pos] = w147.T @ f ---
            plo = big.tile([128, NFP], f32, tag="plo", name="plo")
            phi = big.tile([19, NFP], f32, tag="phi", name="phi")
            nc.vector.memset(plo[:, NF:NFP], 0.0)
            nc.vector.memset(phi[:, NF:NFP], 0.0)
            for k in range(NCHUNK):
                a, b = k * 512, min((k + 1) * 512, NF)
                pa = psa.tile([128, 512], f32, tag="pa", name="pa")
                pb = psh.tile([19, 512], f32, tag="pb", name="pb")
                nc.tensor.matmul(out=pa[:, :b - a], lhsT=wa_t[:],
                                 rhs=f_t[:, a:b], start=True, stop=True)
                nc.tensor.matmul(out=pb[:, :b - a], lhsT=wb_t[:],
                                 rhs=f_t[:, a:b], start=True, stop=True)
                nc.vector.tensor_copy(plo[:, a:b], pa[:, :b - a])
                nc.scalar.copy(phi[:, a:b], pb[:, :b - a])
            # --- tap shift-copies: Psh[r, o] = P[r, o + dy*WP + dx] ---
            pshlo = big.tile([128, ND], f32, tag="imc", name="pshlo")
            pshhi = big.tile([19, ND], f32, tag="f", name="pshhi")
            for t in range(49):
                sft = (t // 7) * WP + (t % 7)
                lo, hi = 3 * t, 3 * t + 3
                if hi <= 128:
                    nc.sync.dma_start(out=pshlo[lo:hi, :],
                                      in_=plo[lo:hi, sft:sft + ND])
                elif lo >= 128:
                    nc.sync.dma_start(out=pshhi[lo - 128:hi - 128, :],
                                      in_=phi[lo - 128:hi - 128, sft:sft + ND])
                else:
                    nc.sync.dma_start(out=pshlo[lo:128, :],
                                      in_=plo[lo:128, sft:sft + ND])
                    nc.sync.dma_start(out=pshhi[0:hi - 128, :],
                                      in_=phi[0:hi - 128, sft:sft + ND])
            # --- selection matmuls: maps[3, o] = sum_r Sel[r,c] Psh[r, o] ---
            out_t = big.tile([3, ND], f32, tag="phi", name="out_t")
            for k in range(NOCHUNK):
                a, b = k * 512, min((k + 1) * 512, ND)
                po = pso.tile([3, 512], f32, tag="po", name="po")
                nc.tensor.matmul(out=po[:, :b - a], lhsT=sl_t[:],
                                 rhs=pshlo[:, a:b], start=True, stop=False)
                nc.tensor.matmul(out=po[:, :b - a], lhsT=sh_t[:],
                                 rhs=pshhi[:, a:b], start=False, stop=True)
                nc.vector.tensor_copy(out_t[:, a:b], po[:, :b - a])
            # --- tanh(ref + b_ref) -> fp16, dump maps ---
            # (channel order is ref_x, ref_y, d: ACT input APs must start
            # at partition 0, so the tanh rows lead)
            tanh_t = big.tile([2, ND], f16, tag="imc", name="tanh_t")
            nc.scalar.activation(tanh_t[:, :], out_t[0:2, :],
                                 mybir.ActivationFunctionType.Tanh,
                                 bias=br_t[:])
            out3 = out_t.rearrange("p (a b) -> p a b", a=SLAB)
            tanh3 = tanh_t.rearrange("p (a b) -> p a b", a=SLAB)
            osl = bass.ds(r0, SLAB) if dyn else slice(r0, r0 + SLAB)
            nc.sync.dma_start(out=dmap_d[osl, :], in_=out3[2:3, :, 0:W])
            nc.sync.dma_start(out=ref_d[:, osl, :], in_=tanh3[:, :, 0:W])

        emit_slab(0, True, False)
        with tc.For_i(SLAB, (NSLAB - 1) * SLAB, SLAB) as r0v:
            emit_slab(r0v, False, False)
        emit_slab((NSLAB - 1) * SLAB, False, True)
    nc.finalize()
    return nc


def _host_canvases(x):
    """Per (image, half): zero-padded canvas [3, CH, CW]; canvas[c, u, v]
    = x[c, h*HALF + u - 4, v - 4] where in-bounds, else 0."""
    xg = np.zeros((B, C_IN, H + 8, W + 8), np.float32)
    xg[:, :, 4:4 + H, 4:4 + W] = x
    out = {}
    for b in range(B):
        for h in range(2):
            out[(b, h)] = np.ascontiguousarray(
                xg[b, :, h * HALF:h * HALF + CH, :])
    return out


_NC_CACHE = None


def kernel(x, w_bb, b_bb, w_score, b_score, w_loc, b_loc,
           w_fourier, b_fourier, w_ref, b_ref):
    import time as _time
    global LAST_EXEC_NS, LAST_DEVICE_S, _NC_CACHE
    t_start = _time.time()
    x = np.asarray(x, np.float32)
    w_bb = np.asarray(w_bb, np.float32)
    w_score = np.asarray(w_score, np.float32)
    w_loc = np.asarray(w_loc, np.float32)
    w_fourier = np.asarray(w_fourier, np.float32)
    w_ref = np.asarray(w_ref, np.float32)
    b_bb = np.asarray(b_bb, np.float32)
    b_ref_np = np.asarray(b_ref, np.float32)

    # ---- weights prep ----
    # backbone lhsT rows ordered (ky, kx, cin) to match the im2col DMAs
    w27 = np.ascontiguousarray(w_bb.transpose(2, 3, 1, 0).reshape(27, C))
    w_d = (w_score[1] - w_score[0]).astype(np.float32)          # [C,7,7]
    whead = np.stack([w_ref[0], w_ref[1], w_d], 0)              # [3,C,7,7]
    w147T = np.ascontiguousarray(
        whead.transpose(2, 3, 0, 1).reshape(147, C))            # rows (dy,dx,c)
    wa = np.ascontiguousarray(w147T[:128].T)                    # [C,128]
    wb = np.ascontiguousarray(w147T[128:].T)                    # [C,19]
    rr = np.arange(147)
    sel = (rr[:, None] % 3 == np.arange(3)[None, :]).astype(np.float32)
    sl = np.ascontiguousarray(sel[:128])
    sh = np.ascontiguousarray(sel[128:])
    br = np.ascontiguousarray(b_ref_np.reshape(2, 1))

    canv = _host_canvases(x)
    TIMINGS['prep'] = _time.time() - t_start

    # ---- device run ----
    from concourse.bass_utils import run_bass_kernel_spmd
    t0 = _time.time()
    if _NC_CACHE is None:
        _NC_CACHE = _build_device_program()
    nc = _NC_CACHE
    TIMINGS['build'] = _time.time() - t0
    mz0 = np.zeros((C, 1), np.float32)
    mz1 = np.ones((C, 1), np.float32)
    in_maps = []
    for core in range(8):
        b, h = core // 2, core % 2
        in_maps.append({"xc": canv[(b, h)], "w27": w27, "wa": wa, "wb": wb,
                        "sl": sl, "sh": sh, "br": br,
                        "mzf": mz0 if h == 0 else mz1,
                        "mzl": mz1 if h == 0 else mz0})
    t0 = _time.time()
    res = run_bass_kernel_spmd(nc, in_maps, core_ids=list(range(8)))
    LAST_DEVICE_S = _time.time() - t0
    TIMINGS['device'] = LAST_DEVICE_S
    LAST_EXEC_NS = res.exec_time_ns

    # ---- host: assemble maps ----
    t0 = _time.time()
    d_map = np.empty((B, H, W), np.float32)
    ref_map = np.empty((B, 2, H, W), np.float32)
    for core in range(8):
        b, h = core // 2, core % 2
        sl_ = slice(h * HALF, (h + 1) * HALF)
        d_map[b, sl_] = res.results[core]["dmap"]
        ref_map[b, :, sl_] = np.float32(MARGIN) * np.asarray(
            res.results[core]["ref"], np.float32)

    # ---- top-k ordering: sigmoid is monotone in d, so order by d directly
    # (value desc, index asc on ties — matches jax.lax.top_k) ----
    dd = d_map.reshape(B, H * W)
    NC_ = 2048
    cand = np.argpartition(-dd, NC_ - 1, axis=1)[:, :NC_]
    cval = np.take_along_axis(dd, cand, 1)
    top_idx = np.empty((B, N_DET), np.int32)
    for b in range(B):
        order = np.lexsort((cand[b], -cval[b]))[:N_DET]
        top_idx[b] = cand[b][order]
    px = (top_idx % W).astype(np.float32)
    py = (top_idx // W).astype(np.float32)

    # ---- loc/fourier head at detections: recompute f patches from x ----
    xp = np.zeros((B, C_IN, H + 8, W + 8), np.float32)
    xp[:, :, 4:4 + H, 4:4 + W] = x
    swx = np.lib.stride_tricks.sliding_window_view(xp, (9, 9), axis=(2, 3))
    # swx[b, c, iy, ix, :, :] = xp[b, c, iy:iy+9, ix:ix+9]
    w27h = w_bb.reshape(C, 27).T                                # rows (cin,ky,kx)
    w22 = np.concatenate([w_loc, w_fourier], 0)                 # [22,C,7,7]
    w22f = w22.reshape(22, C * 49)
    b22 = np.concatenate([np.asarray(b_loc, np.float32),
                          np.asarray(b_fourier, np.float32)], 0)
    head22 = np.empty((B, N_DET, 22), np.float32)
    for b in range(B):
        iy, ix = top_idx[b] // W, top_idx[b] % W
        pat = swx[b][:, iy, ix]                                 # [3,N,9,9]
        pw = np.lib.stride_tricks.sliding_window_view(
            pat, (3, 3), axis=(2, 3))                           # [3,N,7,7,3,3]
        pwr = np.ascontiguousarray(
            pw.transpose(1, 2, 3, 0, 4, 5)).reshape(N_DET * 49, 27)
        fw = np.maximum(pwr @ w27h + b_bb[None, :], 0.0)        # [N*49, C]
        # head conv zero-pads f beyond the image: mask out-of-bounds positions
        off = np.arange(7) - 3
        fy = iy[:, None, None] + off[:, None]                   # [N,7,1]
        fx = ix[:, None, None] + off[None, :]                   # [N,1,7]
        valid = ((fy >= 0) & (fy < H) & (fx >= 0) & (fx < W))   # [N,7,7]
        fw = fw.reshape(N_DET, 49, C) * valid.reshape(N_DET, 49, 1)
        fwr = fw.transpose(0, 2, 1).reshape(N_DET, C * 49)
        head22[b] = fwr @ w22f.T + b22[None, :]

    loc = head22[..., 0:2]
    coef = head22[..., 2:22].reshape(B, N_DET, ORDER, 4)
    cx = (px + loc[..., 0]).astype(np.float32)
    cy = (py + loc[..., 1]).astype(np.float32)

    # ---- fourier contour synthesis ----
    t = np.arange(SAMPLES, dtype=np.float32) / np.float32(SAMPLES)
    kk = np.arange(1, ORDER + 1, dtype=np.float32)
    ang = (np.float32(2.0 * np.pi) * kk[:, None] * t[None, :]).astype(np.float32)
    cos_a = np.cos(ang).astype(np.float32)
    sin_a = np.sin(ang).astype(np.float32)
    xs = (np.einsum("bno,os->bns", coef[..., 0], cos_a, dtype=np.float32)
          + np.einsum("bno,os->bns", coef[..., 1], sin_a, dtype=np.float32)
          + cx[..., None]).astype(np.float32)
    ys = (np.einsum("bno,os->bns", coef[..., 2], cos_a, dtype=np.float32)
          + np.einsum("bno,os->bns", coef[..., 3], sin_a, dtype=np.float32)
          + cy[..., None]).astype(np.float32)
    det = np.stack([xs, ys], -1)

    # ---- refinement iterations ----
    ref_flat = ref_map.reshape(B, 2, H * W)
    for _ in range(ITERS):
        deti = np.round(det)
        xc_ = np.clip(deti[..., 0], 0, W - 1)
        yc_ = np.clip(deti[..., 1], 0, H - 1)
        lin = (yc_.astype(np.int32) * W + xc_.astype(np.int32)).reshape(B, N_DET * SAMPLES)
        rx = np.take_along_axis(ref_flat[:, 0], lin, 1).reshape(B, N_DET, SAMPLES)
        ry = np.take_along_axis(ref_flat[:, 1], lin, 1).reshape(B, N_DET, SAMPLES)
        det = np.stack([(xc_ + rx).astype(np.float32),
                        (yc_ + ry).astype(np.float32)], -1)
    TIMINGS['post'] = _time.time() - t0
    TIMINGS['total'] = _time.time() - t_start
    return det.astype(np.float32)
